# revision 5
# baseline (speedup 1.0000x reference)
"""Trainium2 Bass kernel for nn_AFM_layer (AFM-style pooling model).

Math (from the reference):
    x1 = concat(dense, gather(emb_tables, sparse))            # [B, 221]
    x2 = (x1 (x) x1) @ W1 + b1                                # [B, 221]
    x3 = (x2 (x) x2) @ W2 + b2                                # [B, 221]
    (softmax over a size-1 axis is all-ones, so the "attention" pooling
     reduces to a plain sum over features)
    y  = sigmoid(sum_k(x3) * out_w + out_b)                   # [B, 1]

Device strategy (data-parallel over batch, 8 cores, 256 samples each):
  * Symmetrized pair products: only (i, j>=i) pairs, with W rows
    pre-combined on host (U[(i,j),k] = W3[i,j,k]+W3[j,i,k], diag once).
  * Pairs are built in fp8e4m3 (inputs pre-scaled so emb*emb products sit
    in fp8's normal range; the scales are divided out of U on the host).
    Build ops (tensor_scalar with per-partition scalar) are split across
    DVE + ACT (+ Pool for layer 2).
  * The batch-major fp8 pair matrix is transposed through the DMA xbar
    bitcast as fp16, which halves the packet count vs fp16 pairs AND
    lands pair rows two-to-a-partition in exactly the interleaved layout
    the PE's DoubleRowSwInterleave fp8 matmul mode expects (256 pairs of
    contraction per 128-partition tile, 0.5 cycles/row).  Each layer's
    matmul chain makes psum rows come out sample-REVERSED; two layers
    cancel, so y is in natural order.
  * Embedding gather: 52 serial indirect DMAs on the Pool queue (HW
    supports one gather descriptor per partition per instruction),
    issued field-descending and interleaved with the build groups so
    high-i pair builds start while low fields still gather.
  * Group sizes are progressive (small first) so the build->transpose->
    matmul pipeline warms up quickly.
"""

import sys

if "/opt/trn_rl_repo" not in sys.path:
    sys.path.insert(0, "/opt/trn_rl_repo")

import numpy as np
import ml_dtypes

B, D, S, V, E = 2048, 13, 26, 100000, 8
F = D + S * E  # 221
N_CORES = 8
BC = B // N_CORES  # 256 samples per core
NT = BC // 128  # batch tiles per core
FPAD = 224

S_EMB = 16.0  # fp8 range scale for embedding features
S_DENSE = 4.0  # scalar-side scale for dense features (keeps dd pairs < fp8 max)
S_X2 = 8.0  # fp8 range scale for layer-2 activations

# Entries processed descending i; entry i covers pair columns
# [col(i), col(i)+w) with w = F - i, pairs (i, j=i..F-1).
ENTRIES = []  # (i, col, w)
_col = 0
for _i in range(F - 1, -1, -1):
    ENTRIES.append((_i, _col, F - _i))
    _col += F - _i
NP_RAW = _col  # 24531
NP = -(-NP_RAW // 256) * 256  # 24576
NBLK = NP // 256  # 96

# Progressive group sizes (in 256-pair blocks): small first for fast
# pipeline warmup while gathers are still landing.
GROUP_BLOCKS = [2, 4, 8, 12, 16, 16, 19, 19]
assert sum(GROUP_BLOCKS) == NBLK


def build_groups():
    """Split ENTRIES into column groups; entries straddling a boundary are
    split into pieces.  Returns list of (pieces, col0, ncols, blk0) where
    pieces = [(i, j0, colg, w)]: build writes group-local cols
    [colg, colg+w) = x[:, j0:j0+w] * x[:, i]."""
    groups = []
    bounds = []
    c = 0
    for nb in GROUP_BLOCKS:
        bounds.append((c, c + nb * 256))
        c += nb * 256
    ei = 0
    off = 0  # consumed width of current entry
    blk0 = 0
    for gcol0, gcol1 in bounds:
        pieces = []
        c = gcol0
        while c < gcol1 and ei < len(ENTRIES):
            i, col, w = ENTRIES[ei]
            take = min(w - off, gcol1 - c)
            pieces.append((i, i + off, c - gcol0, take))
            off += take
            c += take
            if off == w:
                ei += 1
                off = 0
        groups.append((pieces, gcol0, gcol1 - gcol0, blk0))
        blk0 += (gcol1 - gcol0) // 256
    return groups


GROUPS = build_groups()


def feat_scales():
    s = np.ones(F, np.float32)
    s[D:] = S_EMB
    return s


def pack_u(
    w_mat: np.ndarray, scal_scale: np.ndarray, vec_scale: np.ndarray
) -> tuple[np.ndarray, float]:
    """Pack [F*F, F] weights into the fp8 DoubleRowSwInterleave layout
    [128, NBLK, 2, F] (uint8 view).  Row (ij) is divided by
    scal_scale[i]*vec_scale[j] (the build-side scaling: pair'(i,j) =
    (scal_scale[i] x_i)(vec_scale[j] x_j)) and multiplied by a global gain
    gamma chosen so the fp8 values use the format's range; gamma is
    returned so the epilogue can divide it back out."""
    w3 = w_mat.reshape(F, F, F)
    u = np.zeros((NP, F), np.float32)
    for i, col, w in ENTRIES:
        blk = w3[i, i:F, :] + w3[i:F, i, :]  # [w, F]
        blk[0] = w3[i, i, :]
        u[col : col + w] = blk / (scal_scale[i] * vec_scale[i:F, None])
    gamma = 160.0 / max(1e-30, float(np.abs(u).max()))
    u8 = (u * gamma).astype(ml_dtypes.float8_e4m3)
    # u[blk*256 + 2p + s] -> out[p, blk, s]
    out = np.ascontiguousarray(
        u8.reshape(NBLK, 128, 2, F).transpose(1, 0, 2, 3)
    )
    return out.view(np.uint8), gamma


_COMPILED = None


def _build_kernel():
    import concourse.bass as bass
    import concourse.mybir as mybir
    import concourse.tile as tile
    from concourse import bacc

    dt = mybir.dt
    f32, f16, i32 = dt.float32, dt.float16, dt.int32
    f8 = dt.float8e4

    nc = bacc.Bacc("TRN2", target_bir_lowering=False, debug=True)

    dense = nc.declare_dram_parameter("dense", [BC, D], f32, isOutput=False)
    gidx = nc.declare_dram_parameter("gidx", [128, NT, S], i32, isOutput=False)
    emb2d = nc.declare_dram_parameter("emb2d", [S * V, E], f32, isOutput=False)
    u1 = nc.declare_dram_parameter("u1", [128, NBLK, 2, F], f8, isOutput=False)
    u2 = nc.declare_dram_parameter("u2", [128, NBLK, 2, F], f8, isOutput=False)
    b1rs = nc.declare_dram_parameter("b1rs", [128, F], f32, isOutput=False)
    pb2 = nc.declare_dram_parameter("pb2", [128, 1], f32, isOutput=False)
    esc1 = nc.declare_dram_parameter("esc1", [128, 1], f32, isOutput=False)
    esc2 = nc.declare_dram_parameter("esc2", [128, 1], f32, isOutput=False)
    y = nc.declare_dram_parameter("y", [BC, 1], f32, isOutput=True)

    max_ncols = max(g[2] for g in GROUPS)
    max_nblk = max_ncols // 256

    # field s covers features [D+8s, D+8s+8); builds for min feature i need
    # all fields >= fld(i)
    def fld(i):
        return 0 if i < D else (i - D) // E

    def assign_engines(pieces, engines):
        """Greedy assignment of build pieces to engines by accumulated cost.
        engines: list of (name, fixed_ns, per_col_ns)."""
        acc = [0.0] * len(engines)
        out = []
        for p in pieces:
            w = p[3]
            best, bcost = 0, None
            for k, (nm, fx, pc) in enumerate(engines):
                c = acc[k] + fx + pc * w
                if bcost is None or c < bcost:
                    best, bcost = k, c
            acc[best] += engines[best][1] + engines[best][2] * w
            out.append(engines[best][0])
        return out

    ENG_L1 = [("v", 60.0, 1.042), ("a", 185.0, 0.833)]
    ENG_L2 = [("v", 60.0, 1.042), ("a", 185.0, 0.833), ("p", 95.0, 1.39)]

    with tile.TileContext(nc) as tc:
        with (
            tc.tile_pool(name="persist", bufs=1) as persist,
            tc.tile_pool(name="pair", bufs=4) as pair_pool,
            tc.tile_pool(name="pairt", bufs=4) as pairt_pool,
            tc.tile_pool(name="upool", bufs=3) as upool,
            tc.tile_pool(name="psum", bufs=2, space="PSUM") as psum_pool,
            tc.tile_pool(name="tail", bufs=2) as tail_pool,
        ):
            b1rs_sb = persist.tile([128, F], f32)
            nc.sync.dma_start(b1rs_sb[:], b1rs[:])
            pb2_sb = persist.tile([128, 1], f32)
            nc.sync.dma_start(pb2_sb[:], pb2[:])
            esc1_sb = persist.tile([128, 1], f32)
            nc.sync.dma_start(esc1_sb[:], esc1[:])
            esc2_sb = persist.tile([128, 1], f32)
            nc.sync.dma_start(esc2_sb[:], esc2[:])
            gidx_sb = persist.tile([128, NT, S], i32)
            nc.sync.dma_start(gidx_sb[:], gidx[:])

            # Pre-warm the sigmoid ACT table.
            warm = persist.tile([128, 1], f32)
            nc.scalar.activation(
                warm[:], pb2_sb[:], mybir.ActivationFunctionType.Sigmoid
            )

            xf = []  # raw f32 (gather target)
            xfs = []  # scaled f32 (build scalars)
            xh = []  # scaled fp16 (build vectors)
            for t in range(NT):
                a = persist.tile([128, FPAD], f32, name=f"xf{t}")
                b = persist.tile([128, FPAD], f32, name=f"xfs{t}")
                c = persist.tile([128, FPAD], f16, name=f"xh{t}")
                nc.sync.dma_start(a[:, 0:D], dense[t * 128 : (t + 1) * 128, :])
                nc.vector.tensor_scalar_mul(b[:, 0:D], a[:, 0:D], S_DENSE)
                nc.vector.tensor_copy(c[:, 0:D], a[:, 0:D])
                xf.append(a)
                xfs.append(b)
                xh.append(c)

            def emit_gathers(s_lo, cursor):
                """Emit gathers+casts for fields [s_lo, cursor) descending."""
                for s in range(cursor - 1, s_lo - 1, -1):
                    c0 = D + E * s
                    for t in range(NT):
                        nc.gpsimd.indirect_dma_start(
                            out=xf[t][:, c0 : c0 + E],
                            out_offset=None,
                            in_=emb2d[:],
                            in_offset=bass.IndirectOffsetOnAxis(
                                ap=gidx_sb[:, t, s : s + 1], axis=0
                            ),
                        )
                        nc.vector.tensor_scalar_mul(
                            xfs[t][:, c0 : c0 + E], xf[t][:, c0 : c0 + E], S_EMB
                        )
                        nc.vector.tensor_scalar_mul(
                            xh[t][:, c0 : c0 + E], xf[t][:, c0 : c0 + E], S_EMB
                        )
                return s_lo

            cursor = S  # fields >= cursor already gathered

            for L in range(2):
                u_dram = u1 if L == 0 else u2
                engines = ENG_L1 if L == 0 else ENG_L2
                psum_acc = [
                    psum_pool.tile([128, F], f32, tag=f"acc{t}", name=f"acc{L}_{t}")
                    for t in range(NT)
                ]
                nblk_done = 0
                for gi, (pieces, col0, ncols, blk0) in enumerate(GROUPS):
                    if L == 0:
                        min_i = min(p[0] for p in pieces)
                        cursor = emit_gathers(fld(min_i), cursor)
                    nblk = ncols // 256
                    ug = upool.tile([128, max_nblk, 2, F], f8, tag="ug")
                    nc.sync.dma_start(
                        ug[:, 0:nblk, :, :], u_dram[:, blk0 : blk0 + nblk, :, :]
                    )
                    eng = assign_engines(pieces, engines)
                    for t in range(NT):
                        pb = pair_pool.tile([128, max_ncols], f8, tag="pair")
                        if gi == len(GROUPS) - 1 and NP_RAW < NP:
                            nc.vector.memset(
                                pb[:, NP_RAW - col0 : NP - col0], 0.0
                            )
                        for (i, j0, cg, w), e in zip(pieces, eng):
                            if e == "v":
                                nc.vector.tensor_scalar_mul(
                                    pb[:, cg : cg + w],
                                    xh[t][:, j0 : j0 + w],
                                    xfs[t][:, i : i + 1],
                                )
                            elif e == "a":
                                nc.scalar.activation(
                                    pb[:, cg : cg + w],
                                    xh[t][:, j0 : j0 + w],
                                    mybir.ActivationFunctionType.Copy,
                                    scale=xfs[t][:, i : i + 1],
                                )
                            else:
                                nc.gpsimd.tensor_scalar_mul(
                                    pb[:, cg : cg + w],
                                    xh[t][:, j0 : j0 + w],
                                    xfs[t][:, i : i + 1],
                                )
                        pT = pairt_pool.tile([128, max_nblk, 256], f8, tag="pT")
                        nc.sync.dma_start_transpose(
                            pT[:, 0:nblk, :].bitcast(f16),
                            pb[:, 0:ncols].bitcast(f16),
                        )
                        for blk in range(nblk):
                            nc.tensor.matmul(
                                psum_acc[t][:],
                                lhsT=pT[:, blk, :],
                                rhs=ug[:, blk, :, :],
                                start=(nblk_done + blk == 0),
                                stop=(nblk_done + blk == NBLK - 1),
                                perf_mode=mybir.MatmulPerfMode.DoubleRowSwInterleave,
                            )
                    nblk_done += nblk

                for t in range(NT):
                    if L == 0:
                        # x2_scaled = psum * S_X2 + b1 * S_X2   (rows are
                        # sample-reversed; consistent through layer 2)
                        nc.vector.tensor_scalar_mul(
                            xfs[t][:, 0:F], psum_acc[t][:], esc1_sb[:, 0:1]
                        )
                        nc.vector.tensor_add(
                            xfs[t][:, 0:F], xfs[t][:, 0:F], b1rs_sb[:]
                        )
                        nc.vector.tensor_copy(xh[t][:, 0:F], xfs[t][:, 0:F])
                    else:
                        pooled = tail_pool.tile([128, 1], f32, tag=f"pool{t}")
                        nc.vector.tensor_reduce(
                            pooled[:],
                            psum_acc[t][:],
                            axis=mybir.AxisListType.X,
                            op=mybir.AluOpType.add,
                        )
                        pooled2 = tail_pool.tile([128, 1], f32, tag=f"pool2{t}")
                        nc.vector.tensor_scalar_mul(
                            pooled2[:], pooled[:], esc2_sb[:, 0:1]
                        )
                        yt = tail_pool.tile([128, 1], f32, tag=f"yt{t}")
                        nc.scalar.activation(
                            yt[:],
                            pooled2[:],
                            mybir.ActivationFunctionType.Sigmoid,
                            bias=pb2_sb[:, 0:1],
                            scale=1.0,
                        )
                        nc.sync.dma_start(y[t * 128 : (t + 1) * 128, :], yt[:])

    nc.compile()
    return nc


def _get_compiled():
    global _COMPILED
    if _COMPILED is None:
        _COMPILED = _build_kernel()
    return _COMPILED


def make_in_maps(dense_inputs, sparse_inputs, emb_tables, W1, b1, W2, b2, out_w, out_b):
    dense_inputs = np.asarray(dense_inputs, np.float32)
    sparse_inputs = np.asarray(sparse_inputs, np.int32)
    emb_tables = np.asarray(emb_tables, np.float32)
    ow = float(np.asarray(out_w).reshape(-1)[0])
    ob = float(np.asarray(out_b).reshape(-1)[0])

    emb2d = np.ascontiguousarray(emb_tables.reshape(S * V, E))
    gidx_all = (
        sparse_inputs + (np.arange(S, dtype=np.int32) * V)[None, :]
    ).astype(np.int32)

    # layer 1: pair'(i,j) = (t_i x_i) * (s_j x_j),
    # t_i = 4 dense / 16 emb (scalar side), s_j = 1 dense / 16 emb (vector)
    scal1 = np.where(np.arange(F) < D, S_DENSE, S_EMB).astype(np.float32)
    u1, gamma1 = pack_u(np.asarray(W1, np.float32), scal1, feat_scales())
    # layer 2: pair2' = (8 x2_i) * (8 x2_j)
    sx2 = np.full(F, S_X2, np.float32)
    u2, gamma2 = pack_u(np.asarray(W2, np.float32) * ow, sx2, sx2)
    b1rs = np.ascontiguousarray(
        np.tile(np.asarray(b1, np.float32)[None, :] * S_X2, (128, 1))
    )
    pb2_val = float(np.sum(np.asarray(b2, np.float32)) * ow + ob)
    pb2 = np.full((128, 1), pb2_val, np.float32)
    esc1 = np.full((128, 1), S_X2 / gamma1, np.float32)
    esc2 = np.full((128, 1), 1.0 / gamma2, np.float32)

    in_maps = []
    for c in range(N_CORES):
        sl = slice(c * BC, (c + 1) * BC)
        g = gidx_all[sl]  # [BC, S]
        gidx_tiled = np.ascontiguousarray(
            g.reshape(NT, 128, S).transpose(1, 0, 2)
        )
        in_maps.append(
            {
                "dense": np.ascontiguousarray(dense_inputs[sl]),
                "gidx": gidx_tiled,
                "emb2d": emb2d,
                "u1": u1,
                "u2": u2,
                "b1rs": b1rs,
                "pb2": pb2,
                "esc1": esc1,
                "esc2": esc2,
            }
        )
    return in_maps


def kernel(
    dense_inputs,
    sparse_inputs,
    emb_tables,
    W1,
    b1,
    W2,
    b2,
    att_w_w,
    att_w_b,
    att_h_w,
    att_h_b,
    out_w,
    out_b,
):
    from concourse.bass_utils import run_bass_kernel_spmd

    nc = _get_compiled()
    in_maps = make_in_maps(
        dense_inputs, sparse_inputs, emb_tables, W1, b1, W2, b2, out_w, out_b
    )
    res = run_bass_kernel_spmd(nc, in_maps, list(range(N_CORES)))
    y = np.concatenate([res.results[c]["y"] for c in range(N_CORES)], axis=0)
    return y.astype(np.float32)
